# revision 61
# baseline (speedup 1.0000x reference)
"""BiAttention (BiDAF) Trainium2 Bass kernel — 8 NeuronCores, sequence-
parallel over the context axis.

kernel(context [16384,100] f32, question [4096,100] f32, kernel [300] f32)
  -> G [16384, 400] f32  (concat: ctx | U_A | ctx*U_A | ctx*H_A)

Single-S-pass scheme. A host-computed global stability constant B is folded
into the c.w1 bias row, so each PSUM S-chunk already holds S-B and ACT exps
it straight into bf16 ptt (the exact row-max pass of the two-pass scheme is
gone). U_A/Z accumulate on the PE from bf16 ptt at 1 cyc/row. The exact Q2C
row-maxes come from a bf16 running elementwise max over ptt (DVE 2x mode)
folded + PE-transposed per ctx tile: y = exp(m - B), so the Q2C softmax is
just y/sum(y) — no exp/log — and with a global B the cross-core combine
after the 102-float AllGather is a plain 8-row sum.
"""
import sys

sys.path.insert(0, "/opt/trn_rl_repo")
from contextlib import ExitStack

import numpy as np

import concourse.bass as bass
import concourse.tile as tile
from concourse import mybir


def split_multi_waits(nc):
    """This walrus build rejects instructions with >1 sync wait. Hoist extra
    waits onto single-wait EventSemaphore nops on the same engine (engines
    execute in order, so N sequential single waits == one N-way wait)."""
    n_split = 0
    counter = [0]

    def make_nop(engine, wait):
        counter[0] += 1
        inst = mybir.InstEventSemaphore(
            name=f"I-waitsplit-{counter[0]}", ins=[], outs=[])
        inst.engine = engine
        inst.sync_info = mybir.SyncInfo(on_wait=[wait], on_update=[])
        return inst

    for f in nc.m.functions:
        for blk in f.blocks:
            changed = False
            new_insts = []
            for inst in blk.instructions:
                si = inst.sync_info
                if si is not None and si.on_wait and len(si.on_wait) > 1:
                    waits = list(si.on_wait)
                    for w in waits[:-1]:
                        new_insts.append(make_nop(inst.engine, w))
                    si.on_wait = [waits[-1]]
                    n_split += 1
                    changed = True
                new_insts.append(inst)
            if changed:
                blk.instructions[:] = new_insts
    return n_split


F32 = mybir.dt.float32
F32R = mybir.dt.float32r
BF16 = mybir.dt.bfloat16
EXP = mybir.ActivationFunctionType.Exp
COPY = mybir.ActivationFunctionType.Copy

N_CORES = 8
D = 100
R = 2048          # ctx rows per core
M = 4096          # question rows
P = 128           # partitions
NCH = R // P      # 16 ctx chunks (natural layout)
QC = M // P       # 32 q chunks
TW = 512          # ctx tile width
NT = R // TW      # 4 ctx tiles
CPT = TW // P     # 4 ctx chunks per tile
GRP = 3           # q-chunks per exp group (3 PSUM banks)


def build_bass():
    nc = bass.Bass("TRN2", target_bir_lowering=False, debug=False,
                   num_devices=N_CORES)
    ctx_in = nc.dram_tensor("ctx", [R, D], F32, kind="ExternalInput").ap()
    ctxTa_in = nc.dram_tensor("ctxTa", [102, R], F32, kind="ExternalInput").ap()
    qaugTa_in = nc.dram_tensor("qaugTa", [102, M], F32, kind="ExternalInput").ap()
    qnr_in = nc.dram_tensor("qnr", [M, D], BF16, kind="ExternalInput").ap()
    id_in = nc.dram_tensor("ident", [P, P], F32, kind="ExternalInput").ap()
    g_out = nc.dram_tensor("g", [R, 4 * D], F32, kind="ExternalOutput").ap()

    with tile.TileContext(nc) as tc:
        with ExitStack() as ex:
            build_body(nc, tc, ex, ctx_in, ctxTa_in, qaugTa_in, qnr_in,
                       id_in, g_out)
    return nc


def build_body(nc, tc, ex, ctx_in, ctxTa_in, qaugTa_in, qnr_in, id_in, g_out):
    sing = ex.enter_context(tc.tile_pool(name="sing", bufs=1))
    ptt_pool = ex.enter_context(tc.tile_pool(name="ptt", bufs=4))
    m1_pool = ex.enter_context(tc.tile_pool(name="m1", bufs=2))
    uat_pool = ex.enter_context(tc.tile_pool(name="uat", bufs=2))
    g12_pool = ex.enter_context(tc.tile_pool(name="g12", bufs=3))
    # PSUM: stp 2x[128,1536]f32 (6 banks) + UA [101,512] (1) + tiny (1) = 8
    stp = ex.enter_context(tc.tile_pool(name="stp", bufs=2, space="PSUM"))
    uap = ex.enter_context(tc.tile_pool(name="uap", bufs=1, space="PSUM"))
    tp = ex.enter_context(tc.tile_pool(name="tp", bufs=1, space="PSUM"))
    dram = ex.enter_context(tc.tile_pool(name="dram", bufs=1, space="DRAM"))

    # ---- persistent SBUF ----
    caugT = sing.tile([102, R], F32R)  # 0..99 ctxT | 100 ones | 101 c.w1 - B
    qaugT = sing.tile([102, M], F32R)  # 0..99 qT*w3 | 100 q.w2 | 101 ones
    stg_c = sing.tile([102, R], F32)
    stg_q = sing.tile([102, M], F32)
    qaugN = sing.tile([P, QC, 104], BF16)  # q natural chunks + ones col @100
    ctxn = sing.tile([P, NCH, 104], F32)   # ctx natural chunks + ones col @100
    tid = sing.tile([P, P], F32)
    ystore = sing.tile([P, NCH], F32)      # y = exp(rowmax - B), natural
    uan = sing.tile([P, NCH, 104], F32)    # U_A unnorm natural + Z col @100
    rzs = sing.tile([P, NCH], F32)         # 1/Z per chunk
    ones1 = sing.tile([1, P], F32)
    ones81 = sing.tile([N_CORES, 1], F32)
    hlacc = sing.tile([101, 1], F32)
    hltmp = sing.tile([101, 1], F32)
    hlrow = sing.tile([1, 101], F32)
    hB = sing.tile([P, D], F32)
    g3big = sing.tile([P, NCH, D], F32)
    dummy = sing.tile([1, 1], F32)

    cc_in = dram.tile([1, 102], F32)
    cc_out = dram.tile([N_CORES, 102], F32)
    cc_w_in = dram.tile([1, 8], F32)
    cc_w_out = dram.tile([N_CORES, 8], F32)
    cc_m_in = dram.tile([1, 8], F32)
    cc_m_out = dram.tile([N_CORES, 8], F32)

    # ---- input loads (critical first: qaugT piece 0 + caugT tile 0) ----
    # f32r matmul inputs must be produced by a rounding instruction, so DMA
    # lands in f32 staging and ACT/DVE copy-round into the f32r tiles. The
    # two pieces that gate the pipeline start go on ACT, the rest on DVE.
    # Bulk, non-gating loads trigger from the idle Pool queue.
    nc.sync.dma_start(out=stg_c[:, 0:TW], in_=ctxTa_in[:, 0:TW])
    nc.sync.dma_start(out=stg_q[:, 0:384], in_=qaugTa_in[:, 0:384])
    nc.sync.dma_start(out=stg_q[:, 384:1024], in_=qaugTa_in[:, 384:1024])
    nc.vector.memset(dummy[:], 0.0)
    # preload the exp table set early (hidden behind input DMAs); keep the
    # ACT queue free of DMA triggers so the gating copies run ASAP
    nc.scalar.activation(dummy[:], dummy[:], EXP)
    nc.scalar.activation(caugT[:, 0:TW], stg_c[:, 0:TW], COPY)
    nc.scalar.activation(qaugT[:, 0:384], stg_q[:, 0:384], COPY)
    # q640 copy goes on DVE: on ACT it would queue ahead of exp(0) and gate
    # the whole pipeline on its (late) DMA
    nc.vector.tensor_copy(qaugT[:, 384:1024], stg_q[:, 384:1024])
    # gating loads first: qaugT pieces + caugT tiles feed the S pipeline
    # directly; bulk non-gating loads (qaugN/tid/ctxn/g0) are held behind
    # the last staging copy so their transfers don't hog the DMA engines.
    p3copy = p1dma = None
    for piece in range(1, 4):
        pdma = nc.sync.dma_start(
            out=stg_q[:, piece * 1024:(piece + 1) * 1024],
            in_=qaugTa_in[:, piece * 1024:(piece + 1) * 1024])
        if piece == 1:
            p1dma = pdma
        p3copy = nc.vector.tensor_copy(
            qaugT[:, piece * 1024:(piece + 1) * 1024],
            stg_q[:, piece * 1024:(piece + 1) * 1024])
    for t in range(1, NT):
        nc.sync.dma_start(out=stg_c[:, t * TW:(t + 1) * TW],
                          in_=ctxTa_in[:, t * TW:(t + 1) * TW])
        nc.vector.tensor_copy(caugT[:, t * TW:(t + 1) * TW],
                              stg_c[:, t * TW:(t + 1) * TW])
    from concourse.tile_rust import add_dep_helper as _adh
    nc.vector.memset(qaugN[:, :, 100:104], 1.0)
    d_qn = nc.gpsimd.dma_start(
        out=qaugN[:, :, 0:D],
        in_=qnr_in.rearrange("(c p) d -> p c d", p=P))
    # hold qaugN's bulk transfer behind the first qaugT piece so the piece
    # staging (which gates S groups 3+) isn't crowded off the DMA engines;
    # qaugN itself is only needed by UA(0) a few groups later
    _adh(d_qn.ins, p1dma.ins, sync=True, reason="qaugN after piece-1 dma")
    d_tid = nc.gpsimd.dma_start(out=tid[:], in_=id_in[:])
    nc.vector.memset(ctxn[:, :, 100:104], 1.0)
    d_ctxn = nc.gpsimd.dma_start(
        out=ctxn[:, :, 0:D],
        in_=ctx_in.rearrange("(c p) d -> p c d", p=P))
    nc.vector.memset(ones1[:], 1.0)
    nc.vector.memset(ones81[:], 1.0)

    # G cols 0:100 = context verbatim (DRAM->DRAM); least urgent load
    d_g0 = nc.gpsimd.dma_start(out=g_out[:, 0:D], in_=ctx_in[:])
    for d in (d_tid, d_ctxn, d_g0):
        _adh(d.ins, p3copy.ins, sync=True, reason="bulk loads after staging")

    # Warm-up AllGather doubling as a cross-core barrier: absorbs NEFF start
    # skew and warms the CC rings so the real end-of-loop collective only
    # pays its intrinsic latency.
    nc.gpsimd.collective_compute(
        "AllGather", mybir.AluOpType.bypass,
        replica_groups=[list(range(N_CORES))],
        ins=[cc_w_in.opt()], outs=[cc_w_out.opt()])

    # q-chunk groups: 10x3 + 1x2
    groups = [list(range(g, min(g + GRP, QC))) for g in range(0, QC, GRP)]

    def q2c_dve(t, m1):
        """Fold the 3-slot running max down to y-per-ctx-col (DVE only)."""
        tmpm = m1_pool.tile([P, TW], BF16, tag="tmpm")
        nc.vector.tensor_max(tmpm[:], m1[:, 0:TW], m1[:, TW:2 * TW])
        yt = m1_pool.tile([P, TW], F32, tag="yt")
        nc.vector.tensor_max(yt[:], tmpm[:], m1[:, 2 * TW:3 * TW])
        return yt

    def ua_evict(t, uaps):
        uat = uat_pool.tile([101, TW], F32, tag="uat")
        nc.vector.tensor_copy(uat[:], uaps[:])
        return uat

    def q2c_pe_a(t, yt):
        """y transpose to natural layout (PE) + column-max reduces."""
        yps4 = tp.tile([P, CPT * P], F32, tag="tiny", name=f"yps4_{t}")
        for ci in range(CPT):
            nc.tensor.transpose(yps4[:, ci * P:(ci + 1) * P],
                                yt[:, ci * P:(ci + 1) * P], tid[:])
        for ci in range(CPT):
            cc = t * CPT + ci
            nc.vector.reduce_max(ystore[:, cc:cc + 1],
                                 yps4[:, ci * P:(ci + 1) * P],
                                 axis=mybir.AxisListType.X)

    def q2c_pe_b(t):
        """hl partial accumulation into hlacc (+ row form for the last)."""
        hlp = tp.tile([101, 1], F32, tag="tiny", name=f"hlp_{t}")
        for ci in range(CPT):
            cc = t * CPT + ci
            nc.tensor.matmul(hlp[:], ctxn[:, cc, 0:101], ystore[:, cc:cc + 1],
                             start=(ci == 0), stop=(ci == CPT - 1))
        if t == 0:
            nc.vector.tensor_copy(hlacc[:], hlp[:])
        else:
            nc.vector.tensor_copy(hltmp[:], hlp[:])
            nc.vector.tensor_add(hlacc[:], hlacc[:], hltmp[:])
        if t == NT - 1:
            # row-form hl so the collective-input DMA is one descriptor
            hlrps = tp.tile([1, 101], F32, tag="tiny", name="hlr")
            nc.tensor.transpose(hlrps[:], hlacc[:], tid[0:101, 0:101])
            nc.vector.tensor_copy(hlrow[:], hlrps[:])

    def ua_norm2(t, uat, half):
        """U_A normalize + G cols 100:300 for 2 of the 4 chunks."""
        first_pool = None
        for ci in range(2 * half, 2 * half + 2):
            cc = t * CPT + ci
            uanps = tp.tile([P, 101], F32, tag="tiny", name=f"uanps_{cc}")
            nc.tensor.transpose(uanps[:], uat[:, ci * P:(ci + 1) * P],
                                tid[0:101, 0:101])
            nc.vector.tensor_copy(uan[:, cc, 0:101], uanps[:])
            nc.vector.reciprocal(rzs[:, cc:cc + 1], uan[:, cc, 100:101])
            g12 = g12_pool.tile([P, 2 * D], F32, tag="g12")
            nc.vector.tensor_scalar_mul(g12[:, 0:D], uan[:, cc, 0:D],
                                        rzs[:, cc:cc + 1])
            g2op = nc.gpsimd.tensor_mul(g12[:, D:2 * D], ctxn[:, cc, 0:D],
                                        g12[:, 0:D])
            if first_pool is None:
                first_pool = g2op
            last = nc.sync.dma_start(out=g_out[cc * P:(cc + 1) * P, D:3 * D],
                                     in_=g12[:])
        return last, first_pool

    # ---- flat cross-tile software pipeline: UA lags S/exp by one group
    # globally (so the PE never blocks on exp, even across tile boundaries),
    # and the previous tile's tail work is spread one slice per group. ----
    slots = [(t, gi, chunks) for t in range(NT)
             for gi, chunks in enumerate(groups)]
    NG = len(groups)
    state = {}   # per-tile m1/uaps/ptts
    prev_tail = None
    t3_last = t3_pool = None

    def emit_ua(k):
        pt, pgi, pchunks = slots[k]
        pptt = state[pt]["ptts"][pgi]
        for j, qc in enumerate(pchunks):
            nc.tensor.matmul(
                state[pt]["uaps"][:], qaugN[:, qc, 0:101],
                pptt[:, j * TW:(j + 1) * TW],
                start=(qc == 0), stop=(qc == QC - 1))

    for k, (t, gi, chunks) in enumerate(slots):
        ctxsl = caugT[:, t * TW:(t + 1) * TW]
        w = len(chunks) * TW
        sp = stp.tile([P, GRP * TW], F32, tag="sp")
        for j, qc in enumerate(chunks):
            nc.tensor.matmul(
                sp[:, j * TW:(j + 1) * TW],
                qaugT[:, qc * P:(qc + 1) * P],
                ctxsl, start=True, stop=True)
        if k > 0:
            emit_ua(k - 1)
        if gi == 0 and t > 0:
            # previous tile's uaps evict + y folds right at the boundary
            pt = t - 1
            uat = ua_evict(pt, state[pt]["uaps"])
            yt = q2c_dve(pt, state[pt]["m1"])
            prev_tail = (pt, yt, uat)
        if gi == 0:
            state[t] = {
                "m1": m1_pool.tile([P, GRP * TW], BF16, tag="m1",
                                   name=f"m1_{t}"),
                "uaps": uap.tile([101, TW], F32, tag="ua",
                                 name=f"uaps_{t}"),
                "ptts": [],
            }
        ptt = ptt_pool.tile([P, GRP * TW], BF16, tag="ptt",
                            name=f"ptt_{t}_{gi}")
        state[t]["ptts"].append(ptt)
        nc.scalar.activation(ptt[:, 0:w], sp[:, 0:w], EXP)
        m1 = state[t]["m1"]
        if gi == 0:
            nc.vector.tensor_copy(m1[:, 0:w], ptt[:, 0:w])
        else:
            nc.vector.tensor_max(m1[:, 0:w], m1[:, 0:w], ptt[:, 0:w])
        if t == 2 and gi == 1:
            # mid-loop barrier: re-aligns the cores so the final collective's
            # peer-data wait only covers tail-of-loop divergence
            nc.gpsimd.collective_compute(
                "AllGather", mybir.AluOpType.bypass,
                replica_groups=[list(range(N_CORES))],
                ins=[cc_m_in.opt()], outs=[cc_m_out.opt()])
        if prev_tail is not None:
            pt, yt, uat = prev_tail
            if gi == 2:
                q2c_pe_a(pt, yt)
            elif gi == 3:
                q2c_pe_b(pt)
            elif gi == 5:
                ua_norm2(pt, uat, 0)
            elif gi == 7:
                ua_norm2(pt, uat, 1)

    emit_ua(len(slots) - 1)
    # last tile tail: folds first so the Q2C/collective chain starts ASAP
    t = NT - 1
    yt = q2c_dve(t, state[t]["m1"])
    uat3 = ua_evict(t, state[t]["uaps"])
    q2c_pe_a(t, yt)
    q2c_pe_b(t)

    # ---- Q2C partials + AllGather (emitted before tile-3's G output so the
    # collective triggers the moment hlacc is ready) ----
    nc.gpsimd.dma_start(out=cc_in[0:1, 0:101], in_=hlrow[:])
    cc_inst = nc.gpsimd.collective_compute(
        "AllGather", mybir.AluOpType.bypass,
        replica_groups=[list(range(N_CORES))],
        ins=[cc_in.opt()], outs=[cc_out.opt()])

    _, t3_pool = ua_norm2(NT - 1, uat3, 0)
    t3_last, _ = ua_norm2(NT - 1, uat3, 1)
    # ordering-only edge: keep the AllGather trigger ahead of tile-3's Pool
    # work (the tile scheduler orders by deps, not emission order)
    _adh(t3_pool.ins, cc_inst.ins, sync=False,
         reason="collective before tile-3 pool muls")

    # ---- combine after AllGather ----
    agm = sing.tile([N_CORES, 102], F32)
    d1 = nc.sync.dma_start(out=agm[:], in_=cc_out[:])
    # keep the combine's load from stalling engines mid-loop
    _adh(d1.ins, t3_last.ins, sync=True, reason="combine after tile3")
    hsps = tp.tile([1, 102], F32, tag="tiny", name="hsps")
    nc.tensor.matmul(hsps[:], ones81[:], agm[:], start=True, stop=True)
    hsum = sing.tile([1, 102], F32)
    nc.scalar.activation(hsum[:], hsps[:], COPY)
    rzh = sing.tile([1, 1], F32)
    nc.vector.reciprocal(rzh[:], hsum[:, 100:101])
    hrow = sing.tile([1, D], F32)
    nc.vector.tensor_scalar_mul(hrow[:], hsum[:, 0:D], rzh[:])
    hbps = tp.tile([P, D], F32, tag="tiny", name="hbps")
    nc.tensor.matmul(hbps[:], ones1[:], hrow[:], start=True, stop=True)
    nc.scalar.activation(hB[:], hbps[:], COPY)
    # 8 half-tile stores alternating Sync/Pool queues so the 0.82MB lands on
    # several DMA engines instead of serializing on one queue
    for half in range(2 * NT):
        for ci in range(2):
            cc = 2 * half + ci
            nc.vector.tensor_mul(g3big[:, cc, :], ctxn[:, cc, 0:D], hB[:])
        eng = nc.sync if half % 2 == 0 else nc.gpsimd
        eng.dma_start(
            out=g_out[half * 256:(half + 1) * 256, 3 * D:4 * D]
            .rearrange("(c p) d -> p c d", p=P),
            in_=g3big[:, 2 * half:2 * half + 2, :])


_nc_cache = None


def _get_nc():
    global _nc_cache
    if _nc_cache is None:
        _nc_cache = build_bass()
        split_multi_waits(_nc_cache)
    return _nc_cache


def _prep_inputs(inputs):
    context = np.ascontiguousarray(inputs["context"], dtype=np.float32)
    question = np.ascontiguousarray(inputs["question"], dtype=np.float32)
    kern = np.ascontiguousarray(inputs["kernel"], dtype=np.float32)
    w1, w2, w3 = kern[:D], kern[D:2 * D], kern[2 * D:]

    # Global stability constant B. exp(S-B) must neither overflow fp32 on
    # the global max nor underflow whole rows to 0 in bf16 (Z=0 -> NaN), so
    # B must sit in [max(S)-80, min_i(rowmax_i)+85]. Row maxes are spread by
    # the per-row c.w1 offset, so estimate them with a q-subsample GEMM and
    # center B in the window.
    cw1 = context @ w1
    q2 = question @ w2
    nq = question.shape[0]
    qsub = question[::16]
    tmax = ((context * w3[None, :]) @ qsub.T + (qsub @ w2)[None, :]).max(axis=1)
    rm = cw1 + tmax
    B = float(0.5 * (rm.max() + rm.min() + 20.0))

    qaugTa = np.empty((102, nq), np.float32)
    qaugTa[0:D] = (question * w3[None, :]).T
    qaugTa[D] = q2
    qaugTa[D + 1] = 1.0
    qaugTa = np.ascontiguousarray(qaugTa)
    import ml_dtypes
    qbf = np.ascontiguousarray(question.astype(ml_dtypes.bfloat16))
    ident = np.eye(P, dtype=np.float32)

    in_maps = []
    for k in range(N_CORES):
        cshard = np.ascontiguousarray(context[k * R:(k + 1) * R])
        ctxTa = np.empty((102, R), np.float32)
        ctxTa[0:D] = cshard.T
        ctxTa[D] = 1.0
        ctxTa[D + 1] = cw1[k * R:(k + 1) * R] - B
        in_maps.append({
            "ctx": cshard,
            "ctxTa": np.ascontiguousarray(ctxTa),
            "qaugTa": qaugTa,
            "qnr": qbf,
            "ident": ident,
        })
    return in_maps


def kernel(**inputs):
    from concourse.bass_utils import run_bass_kernel_spmd

    in_maps = _prep_inputs(inputs)
    res = run_bass_kernel_spmd(_get_nc(), in_maps,
                               core_ids=list(range(N_CORES)))
    return np.concatenate([res.results[k]["g"] for k in range(N_CORES)],
                          axis=0)


def kernel_traced(**inputs):
    """Like kernel() but also returns HW exec time in ns (NTFF profile)."""
    from concourse.bass_utils import run_bass_kernel_spmd

    kernel(**inputs)  # warm compile via cached nc
    in_maps = _prep_inputs(inputs)
    res = run_bass_kernel_spmd(_get_nc(), in_maps,
                               core_ids=list(range(N_CORES)), trace=True)
    out = np.concatenate([res.results[k]["g"] for k in range(N_CORES)],
                         axis=0)
    return out, res


# revision 63
# speedup vs baseline: 1.0681x; 1.0681x over previous
"""BiAttention (BiDAF) Trainium2 Bass kernel — 8 NeuronCores, sequence-
parallel over the context axis.

kernel(context [16384,100] f32, question [4096,100] f32, kernel [300] f32)
  -> G [16384, 400] f32  (concat: ctx | U_A | ctx*U_A | ctx*H_A)

Single-S-pass scheme. A host-computed global stability constant B is folded
into the c.w1 bias row, so each PSUM S-chunk already holds S-B and ACT exps
it straight into bf16 ptt (the exact row-max pass of the two-pass scheme is
gone). U_A/Z accumulate on the PE from bf16 ptt at 1 cyc/row. The exact Q2C
row-maxes come from a bf16 running elementwise max over ptt (DVE 2x mode)
folded + PE-transposed per ctx tile: y = exp(m - B), so the Q2C softmax is
just y/sum(y) — no exp/log — and with a global B the cross-core combine
after the 102-float AllGather is a plain 8-row sum.
"""
import sys

sys.path.insert(0, "/opt/trn_rl_repo")
from contextlib import ExitStack

import numpy as np

import concourse.bass as bass
import concourse.tile as tile
from concourse import mybir


def split_multi_waits(nc):
    """This walrus build rejects instructions with >1 sync wait. Hoist extra
    waits onto single-wait EventSemaphore nops on the same engine (engines
    execute in order, so N sequential single waits == one N-way wait)."""
    n_split = 0
    counter = [0]

    def make_nop(engine, wait):
        counter[0] += 1
        inst = mybir.InstEventSemaphore(
            name=f"I-waitsplit-{counter[0]}", ins=[], outs=[])
        inst.engine = engine
        inst.sync_info = mybir.SyncInfo(on_wait=[wait], on_update=[])
        return inst

    for f in nc.m.functions:
        for blk in f.blocks:
            changed = False
            new_insts = []
            for inst in blk.instructions:
                si = inst.sync_info
                if si is not None and si.on_wait and len(si.on_wait) > 1:
                    waits = list(si.on_wait)
                    for w in waits[:-1]:
                        new_insts.append(make_nop(inst.engine, w))
                    si.on_wait = [waits[-1]]
                    n_split += 1
                    changed = True
                new_insts.append(inst)
            if changed:
                blk.instructions[:] = new_insts
    return n_split


F32 = mybir.dt.float32
F32R = mybir.dt.float32r
BF16 = mybir.dt.bfloat16
EXP = mybir.ActivationFunctionType.Exp
COPY = mybir.ActivationFunctionType.Copy

N_CORES = 8
D = 100
R = 2048          # ctx rows per core
M = 4096          # question rows
P = 128           # partitions
NCH = R // P      # 16 ctx chunks (natural layout)
QC = M // P       # 32 q chunks
TW = 512          # ctx tile width
NT = R // TW      # 4 ctx tiles
CPT = TW // P     # 4 ctx chunks per tile
GRP = 3           # q-chunks per exp group (3 PSUM banks)


def build_bass():
    nc = bass.Bass("TRN2", target_bir_lowering=False, debug=False,
                   num_devices=N_CORES)
    ctx_in = nc.dram_tensor("ctx", [R, D], F32, kind="ExternalInput").ap()
    ctxTa_in = nc.dram_tensor("ctxTa", [102, R], F32, kind="ExternalInput").ap()
    qaugTa_in = nc.dram_tensor("qaugTa", [102, M], F32, kind="ExternalInput").ap()
    qnr_in = nc.dram_tensor("qnr", [M, D], BF16, kind="ExternalInput").ap()
    id_in = nc.dram_tensor("ident", [P, P], F32, kind="ExternalInput").ap()
    g_out = nc.dram_tensor("g", [R, 4 * D], F32, kind="ExternalOutput").ap()

    with tile.TileContext(nc) as tc:
        with ExitStack() as ex:
            build_body(nc, tc, ex, ctx_in, ctxTa_in, qaugTa_in, qnr_in,
                       id_in, g_out)
    return nc


def build_body(nc, tc, ex, ctx_in, ctxTa_in, qaugTa_in, qnr_in, id_in, g_out):
    sing = ex.enter_context(tc.tile_pool(name="sing", bufs=1))
    ptt_pool = ex.enter_context(tc.tile_pool(name="ptt", bufs=4))
    m1_pool = ex.enter_context(tc.tile_pool(name="m1", bufs=2))
    uat_pool = ex.enter_context(tc.tile_pool(name="uat", bufs=2))
    g12_pool = ex.enter_context(tc.tile_pool(name="g12", bufs=3))
    # PSUM: stp 2x[128,1536]f32 (6 banks) + UA [101,512] (1) + tiny (1) = 8
    stp = ex.enter_context(tc.tile_pool(name="stp", bufs=2, space="PSUM"))
    uap = ex.enter_context(tc.tile_pool(name="uap", bufs=1, space="PSUM"))
    tp = ex.enter_context(tc.tile_pool(name="tp", bufs=1, space="PSUM"))
    dram = ex.enter_context(tc.tile_pool(name="dram", bufs=1, space="DRAM"))

    # ---- persistent SBUF ----
    caugT = sing.tile([102, R], F32R)  # 0..99 ctxT | 100 ones | 101 c.w1 - B
    qaugT = sing.tile([102, M], F32R)  # 0..99 qT*w3 | 100 q.w2 | 101 ones
    stg_c = sing.tile([102, R], F32)
    stg_q = sing.tile([102, M], F32)
    qaugN = sing.tile([P, QC, 104], BF16)  # q natural chunks + ones col @100
    ctxn = sing.tile([P, NCH, 104], F32)   # ctx natural chunks + ones col @100
    tid = sing.tile([P, P], F32)
    ystore = sing.tile([P, NCH], F32)      # y = exp(rowmax - B), natural
    uan = sing.tile([P, NCH, 104], F32)    # U_A unnorm natural + Z col @100
    rzs = sing.tile([P, NCH], F32)         # 1/Z per chunk
    ones1 = sing.tile([1, P], F32)
    ones81 = sing.tile([N_CORES, 1], F32)
    hlacc = sing.tile([101, 1], F32)
    hltmp = sing.tile([101, 1], F32)
    hlrow = sing.tile([1, 101], F32)
    hB = sing.tile([P, D], F32)
    g3big = sing.tile([P, NCH, D], F32)
    dummy = sing.tile([1, 1], F32)

    cc_in = dram.tile([1, 102], F32)
    cc_out = dram.tile([N_CORES, 102], F32)
    cc_w_in = dram.tile([1, 8], F32)
    cc_w_out = dram.tile([N_CORES, 8], F32)

    # ---- input loads (critical first: qaugT piece 0 + caugT tile 0) ----
    # f32r matmul inputs must be produced by a rounding instruction, so DMA
    # lands in f32 staging and ACT/DVE copy-round into the f32r tiles. The
    # two pieces that gate the pipeline start go on ACT, the rest on DVE.
    # Bulk, non-gating loads trigger from the idle Pool queue.
    nc.sync.dma_start(out=stg_c[:, 0:TW], in_=ctxTa_in[:, 0:TW])
    nc.sync.dma_start(out=stg_q[:, 0:384], in_=qaugTa_in[:, 0:384])
    nc.sync.dma_start(out=stg_q[:, 384:1024], in_=qaugTa_in[:, 384:1024])
    nc.vector.memset(dummy[:], 0.0)
    # gating copies first on ACT (Copy doesn't need the activation table);
    # the exp-table preload rides after them, still ahead of the first real
    # exp, so neither the table load nor DMA triggers delay pipeline start
    nc.scalar.activation(caugT[:, 0:TW], stg_c[:, 0:TW], COPY)
    nc.scalar.activation(qaugT[:, 0:384], stg_q[:, 0:384], COPY)
    nc.scalar.activation(dummy[:], dummy[:], EXP)
    # q640 copy goes on DVE: on ACT it would queue ahead of exp(0) and gate
    # the whole pipeline on its (late) DMA
    nc.vector.tensor_copy(qaugT[:, 384:1024], stg_q[:, 384:1024])
    # gating loads first: qaugT pieces + caugT tiles feed the S pipeline
    # directly; bulk non-gating loads (qaugN/tid/ctxn/g0) are held behind
    # the last staging copy so their transfers don't hog the DMA engines.
    p3copy = None
    for piece in range(1, 4):
        nc.sync.dma_start(out=stg_q[:, piece * 1024:(piece + 1) * 1024],
                          in_=qaugTa_in[:, piece * 1024:(piece + 1) * 1024])
        p3copy = nc.vector.tensor_copy(
            qaugT[:, piece * 1024:(piece + 1) * 1024],
            stg_q[:, piece * 1024:(piece + 1) * 1024])
    for t in range(1, NT):
        nc.sync.dma_start(out=stg_c[:, t * TW:(t + 1) * TW],
                          in_=ctxTa_in[:, t * TW:(t + 1) * TW])
        nc.vector.tensor_copy(caugT[:, t * TW:(t + 1) * TW],
                              stg_c[:, t * TW:(t + 1) * TW])
    from concourse.tile_rust import add_dep_helper as _adh
    nc.vector.memset(qaugN[:, :, 100:104], 1.0)
    nc.gpsimd.dma_start(
        out=qaugN[:, :, 0:D],
        in_=qnr_in.rearrange("(c p) d -> p c d", p=P))
    d_tid = nc.gpsimd.dma_start(out=tid[:], in_=id_in[:])
    nc.vector.memset(ctxn[:, :, 100:104], 1.0)
    d_ctxn = nc.gpsimd.dma_start(
        out=ctxn[:, :, 0:D],
        in_=ctx_in.rearrange("(c p) d -> p c d", p=P))
    nc.vector.memset(ones1[:], 1.0)
    nc.vector.memset(ones81[:], 1.0)

    # G cols 0:100 = context verbatim (DRAM->DRAM); least urgent load
    d_g0 = nc.gpsimd.dma_start(out=g_out[:, 0:D], in_=ctx_in[:])
    for d in (d_tid, d_ctxn, d_g0):
        _adh(d.ins, p3copy.ins, sync=True, reason="bulk loads after staging")

    # Warm-up AllGather doubling as a cross-core barrier: absorbs NEFF start
    # skew and warms the CC rings so the real end-of-loop collective only
    # pays its intrinsic latency.
    nc.gpsimd.collective_compute(
        "AllGather", mybir.AluOpType.bypass,
        replica_groups=[list(range(N_CORES))],
        ins=[cc_w_in.opt()], outs=[cc_w_out.opt()])

    # q-chunk groups: 10x3 + 1x2
    groups = [list(range(g, min(g + GRP, QC))) for g in range(0, QC, GRP)]

    def q2c_dve(t, m1):
        """Fold the 3-slot running max down to y-per-ctx-col (DVE only)."""
        tmpm = m1_pool.tile([P, TW], BF16, tag="tmpm")
        nc.vector.tensor_max(tmpm[:], m1[:, 0:TW], m1[:, TW:2 * TW])
        yt = m1_pool.tile([P, TW], F32, tag="yt")
        nc.vector.tensor_max(yt[:], tmpm[:], m1[:, 2 * TW:3 * TW])
        return yt

    def ua_evict(t, uaps):
        uat = uat_pool.tile([101, TW], F32, tag="uat")
        nc.vector.tensor_copy(uat[:], uaps[:])
        return uat

    def q2c_pe_a(t, yt):
        """y transpose to natural layout (PE) + column-max reduces."""
        yps4 = tp.tile([P, CPT * P], F32, tag="tiny", name=f"yps4_{t}")
        for ci in range(CPT):
            nc.tensor.transpose(yps4[:, ci * P:(ci + 1) * P],
                                yt[:, ci * P:(ci + 1) * P], tid[:])
        for ci in range(CPT):
            cc = t * CPT + ci
            nc.vector.reduce_max(ystore[:, cc:cc + 1],
                                 yps4[:, ci * P:(ci + 1) * P],
                                 axis=mybir.AxisListType.X)

    def q2c_pe_b(t):
        """hl partial accumulation into hlacc (+ row form for the last)."""
        hlp = tp.tile([101, 1], F32, tag="tiny", name=f"hlp_{t}")
        for ci in range(CPT):
            cc = t * CPT + ci
            nc.tensor.matmul(hlp[:], ctxn[:, cc, 0:101], ystore[:, cc:cc + 1],
                             start=(ci == 0), stop=(ci == CPT - 1))
        if t == 0:
            nc.vector.tensor_copy(hlacc[:], hlp[:])
        else:
            nc.vector.tensor_copy(hltmp[:], hlp[:])
            nc.vector.tensor_add(hlacc[:], hlacc[:], hltmp[:])
        if t == NT - 1:
            # row-form hl so the collective-input DMA is one descriptor
            hlrps = tp.tile([1, 101], F32, tag="tiny", name="hlr")
            nc.tensor.transpose(hlrps[:], hlacc[:], tid[0:101, 0:101])
            nc.vector.tensor_copy(hlrow[:], hlrps[:])

    def ua_norm2(t, uat, half):
        """U_A normalize + G cols 100:300 for 2 of the 4 chunks."""
        first_pool = None
        for ci in range(2 * half, 2 * half + 2):
            cc = t * CPT + ci
            uanps = tp.tile([P, 101], F32, tag="tiny", name=f"uanps_{cc}")
            nc.tensor.transpose(uanps[:], uat[:, ci * P:(ci + 1) * P],
                                tid[0:101, 0:101])
            nc.vector.tensor_copy(uan[:, cc, 0:101], uanps[:])
            nc.vector.reciprocal(rzs[:, cc:cc + 1], uan[:, cc, 100:101])
            g12 = g12_pool.tile([P, 2 * D], F32, tag="g12")
            nc.vector.tensor_scalar_mul(g12[:, 0:D], uan[:, cc, 0:D],
                                        rzs[:, cc:cc + 1])
            g2op = nc.gpsimd.tensor_mul(g12[:, D:2 * D], ctxn[:, cc, 0:D],
                                        g12[:, 0:D])
            if first_pool is None:
                first_pool = g2op
            last = nc.sync.dma_start(out=g_out[cc * P:(cc + 1) * P, D:3 * D],
                                     in_=g12[:])
        return last, first_pool

    # ---- flat cross-tile software pipeline: UA lags S/exp by one group
    # globally (so the PE never blocks on exp, even across tile boundaries),
    # and the previous tile's tail work is spread one slice per group. ----
    slots = [(t, gi, chunks) for t in range(NT)
             for gi, chunks in enumerate(groups)]
    NG = len(groups)
    state = {}   # per-tile m1/uaps/ptts
    prev_tail = None
    t3_last = t3_pool = None

    def emit_ua(k):
        pt, pgi, pchunks = slots[k]
        pptt = state[pt]["ptts"][pgi]
        for j, qc in enumerate(pchunks):
            nc.tensor.matmul(
                state[pt]["uaps"][:], qaugN[:, qc, 0:101],
                pptt[:, j * TW:(j + 1) * TW],
                start=(qc == 0), stop=(qc == QC - 1))

    for k, (t, gi, chunks) in enumerate(slots):
        ctxsl = caugT[:, t * TW:(t + 1) * TW]
        w = len(chunks) * TW
        sp = stp.tile([P, GRP * TW], F32, tag="sp")
        for j, qc in enumerate(chunks):
            nc.tensor.matmul(
                sp[:, j * TW:(j + 1) * TW],
                qaugT[:, qc * P:(qc + 1) * P],
                ctxsl, start=True, stop=True)
        if k > 0:
            emit_ua(k - 1)
        if gi == 0 and t > 0:
            # previous tile's uaps evict + y folds right at the boundary
            pt = t - 1
            uat = ua_evict(pt, state[pt]["uaps"])
            yt = q2c_dve(pt, state[pt]["m1"])
            prev_tail = (pt, yt, uat)
        if gi == 0:
            state[t] = {
                "m1": m1_pool.tile([P, GRP * TW], BF16, tag="m1",
                                   name=f"m1_{t}"),
                "uaps": uap.tile([101, TW], F32, tag="ua",
                                 name=f"uaps_{t}"),
                "ptts": [],
            }
        ptt = ptt_pool.tile([P, GRP * TW], BF16, tag="ptt",
                            name=f"ptt_{t}_{gi}")
        state[t]["ptts"].append(ptt)
        nc.scalar.activation(ptt[:, 0:w], sp[:, 0:w], EXP)
        m1 = state[t]["m1"]
        if gi == 0:
            nc.vector.tensor_copy(m1[:, 0:w], ptt[:, 0:w])
        else:
            nc.vector.tensor_max(m1[:, 0:w], m1[:, 0:w], ptt[:, 0:w])
        if prev_tail is not None:
            pt, yt, uat = prev_tail
            if gi == 2:
                q2c_pe_a(pt, yt)
            elif gi == 3:
                q2c_pe_b(pt)
            elif gi == 5:
                ua_norm2(pt, uat, 0)
            elif gi == 7:
                ua_norm2(pt, uat, 1)

    emit_ua(len(slots) - 1)
    # last tile tail: folds first so the Q2C/collective chain starts ASAP
    t = NT - 1
    yt = q2c_dve(t, state[t]["m1"])
    uat3 = ua_evict(t, state[t]["uaps"])
    q2c_pe_a(t, yt)
    q2c_pe_b(t)

    # ---- Q2C partials + AllGather (emitted before tile-3's G output so the
    # collective triggers the moment hlacc is ready) ----
    nc.gpsimd.dma_start(out=cc_in[0:1, 0:101], in_=hlrow[:])
    cc_inst = nc.gpsimd.collective_compute(
        "AllGather", mybir.AluOpType.bypass,
        replica_groups=[list(range(N_CORES))],
        ins=[cc_in.opt()], outs=[cc_out.opt()])

    _, t3_pool = ua_norm2(NT - 1, uat3, 0)
    t3_last, _ = ua_norm2(NT - 1, uat3, 1)
    # ordering-only edge: keep the AllGather trigger ahead of tile-3's Pool
    # work (the tile scheduler orders by deps, not emission order)
    _adh(t3_pool.ins, cc_inst.ins, sync=False,
         reason="collective before tile-3 pool muls")

    # ---- combine after AllGather ----
    agm = sing.tile([N_CORES, 102], F32)
    d1 = nc.sync.dma_start(out=agm[:], in_=cc_out[:])
    # keep the combine's load from stalling engines mid-loop
    _adh(d1.ins, t3_last.ins, sync=True, reason="combine after tile3")
    hsps = tp.tile([1, 102], F32, tag="tiny", name="hsps")
    nc.tensor.matmul(hsps[:], ones81[:], agm[:], start=True, stop=True)
    hsum = sing.tile([1, 102], F32)
    nc.scalar.activation(hsum[:], hsps[:], COPY)
    rzh = sing.tile([1, 1], F32)
    nc.vector.reciprocal(rzh[:], hsum[:, 100:101])
    hrow = sing.tile([1, D], F32)
    nc.vector.tensor_scalar_mul(hrow[:], hsum[:, 0:D], rzh[:])
    hbps = tp.tile([P, D], F32, tag="tiny", name="hbps")
    nc.tensor.matmul(hbps[:], ones1[:], hrow[:], start=True, stop=True)
    nc.scalar.activation(hB[:], hbps[:], COPY)
    for t in range(NT):
        for ci in range(CPT):
            cc = t * CPT + ci
            nc.vector.tensor_mul(g3big[:, cc, :], ctxn[:, cc, 0:D], hB[:])
        nc.sync.dma_start(
            out=g_out[t * TW:(t + 1) * TW, 3 * D:4 * D]
            .rearrange("(c p) d -> p c d", p=P),
            in_=g3big[:, t * CPT:(t + 1) * CPT, :])


_nc_cache = None


def _get_nc():
    global _nc_cache
    if _nc_cache is None:
        _nc_cache = build_bass()
        split_multi_waits(_nc_cache)
    return _nc_cache


def _prep_inputs(inputs):
    context = np.ascontiguousarray(inputs["context"], dtype=np.float32)
    question = np.ascontiguousarray(inputs["question"], dtype=np.float32)
    kern = np.ascontiguousarray(inputs["kernel"], dtype=np.float32)
    w1, w2, w3 = kern[:D], kern[D:2 * D], kern[2 * D:]

    # Global stability constant B. exp(S-B) must neither overflow fp32 on
    # the global max nor underflow whole rows to 0 in bf16 (Z=0 -> NaN), so
    # B must sit in [max(S)-80, min_i(rowmax_i)+85]. Row maxes are spread by
    # the per-row c.w1 offset, so estimate them with a q-subsample GEMM and
    # center B in the window.
    cw1 = context @ w1
    q2 = question @ w2
    nq = question.shape[0]
    qsub = question[::16]
    tmax = ((context * w3[None, :]) @ qsub.T + (qsub @ w2)[None, :]).max(axis=1)
    rm = cw1 + tmax
    B = float(0.5 * (rm.max() + rm.min() + 20.0))

    qaugTa = np.empty((102, nq), np.float32)
    qaugTa[0:D] = (question * w3[None, :]).T
    qaugTa[D] = q2
    qaugTa[D + 1] = 1.0
    qaugTa = np.ascontiguousarray(qaugTa)
    import ml_dtypes
    qbf = np.ascontiguousarray(question.astype(ml_dtypes.bfloat16))
    ident = np.eye(P, dtype=np.float32)

    in_maps = []
    for k in range(N_CORES):
        cshard = np.ascontiguousarray(context[k * R:(k + 1) * R])
        ctxTa = np.empty((102, R), np.float32)
        ctxTa[0:D] = cshard.T
        ctxTa[D] = 1.0
        ctxTa[D + 1] = cw1[k * R:(k + 1) * R] - B
        in_maps.append({
            "ctx": cshard,
            "ctxTa": np.ascontiguousarray(ctxTa),
            "qaugTa": qaugTa,
            "qnr": qbf,
            "ident": ident,
        })
    return in_maps


def kernel(**inputs):
    from concourse.bass_utils import run_bass_kernel_spmd

    in_maps = _prep_inputs(inputs)
    res = run_bass_kernel_spmd(_get_nc(), in_maps,
                               core_ids=list(range(N_CORES)))
    return np.concatenate([res.results[k]["g"] for k in range(N_CORES)],
                          axis=0)


def kernel_traced(**inputs):
    """Like kernel() but also returns HW exec time in ns (NTFF profile)."""
    from concourse.bass_utils import run_bass_kernel_spmd

    kernel(**inputs)  # warm compile via cached nc
    in_maps = _prep_inputs(inputs)
    res = run_bass_kernel_spmd(_get_nc(), in_maps,
                               core_ids=list(range(N_CORES)), trace=True)
    out = np.concatenate([res.results[k]["g"] for k in range(N_CORES)],
                         axis=0)
    return out, res


# revision 65
# speedup vs baseline: 1.1020x; 1.0317x over previous
"""BiAttention (BiDAF) Trainium2 Bass kernel — 8 NeuronCores, sequence-
parallel over the context axis.

kernel(context [16384,100] f32, question [4096,100] f32, kernel [300] f32)
  -> G [16384, 400] f32  (concat: ctx | U_A | ctx*U_A | ctx*H_A)

Single-S-pass scheme. A host-computed global stability constant B is folded
into the c.w1 bias row, so each PSUM S-chunk already holds S-B and ACT exps
it straight into bf16 ptt (the exact row-max pass of the two-pass scheme is
gone). U_A/Z accumulate on the PE from bf16 ptt at 1 cyc/row. The exact Q2C
row-maxes come from a bf16 running elementwise max over ptt (DVE 2x mode)
folded + PE-transposed per ctx tile: y = exp(m - B), so the Q2C softmax is
just y/sum(y) — no exp/log — and with a global B the cross-core combine
after the 102-float AllGather is a plain 8-row sum.
"""
import sys

sys.path.insert(0, "/opt/trn_rl_repo")
from contextlib import ExitStack

import numpy as np

import concourse.bass as bass
import concourse.tile as tile
from concourse import mybir


def split_multi_waits(nc):
    """This walrus build rejects instructions with >1 sync wait. Hoist extra
    waits onto single-wait EventSemaphore nops on the same engine (engines
    execute in order, so N sequential single waits == one N-way wait)."""
    n_split = 0
    counter = [0]

    def make_nop(engine, wait):
        counter[0] += 1
        inst = mybir.InstEventSemaphore(
            name=f"I-waitsplit-{counter[0]}", ins=[], outs=[])
        inst.engine = engine
        inst.sync_info = mybir.SyncInfo(on_wait=[wait], on_update=[])
        return inst

    for f in nc.m.functions:
        for blk in f.blocks:
            changed = False
            new_insts = []
            for inst in blk.instructions:
                si = inst.sync_info
                if si is not None and si.on_wait and len(si.on_wait) > 1:
                    waits = list(si.on_wait)
                    for w in waits[:-1]:
                        new_insts.append(make_nop(inst.engine, w))
                    si.on_wait = [waits[-1]]
                    n_split += 1
                    changed = True
                new_insts.append(inst)
            if changed:
                blk.instructions[:] = new_insts
    return n_split


F32 = mybir.dt.float32
F32R = mybir.dt.float32r
BF16 = mybir.dt.bfloat16
EXP = mybir.ActivationFunctionType.Exp
COPY = mybir.ActivationFunctionType.Copy

N_CORES = 8
D = 100
R = 2048          # ctx rows per core
M = 4096          # question rows
P = 128           # partitions
NCH = R // P      # 16 ctx chunks (natural layout)
QC = M // P       # 32 q chunks
TW = 512          # ctx tile width
NT = R // TW      # 4 ctx tiles
CPT = TW // P     # 4 ctx chunks per tile
GRP = 3           # q-chunks per exp group (3 PSUM banks)


def build_bass():
    nc = bass.Bass("TRN2", target_bir_lowering=False, debug=False,
                   num_devices=N_CORES)
    ctx_in = nc.dram_tensor("ctx", [R, D], F32, kind="ExternalInput").ap()
    ctxTa_in = nc.dram_tensor("ctxTa", [102, R], F32, kind="ExternalInput").ap()
    qaugTa_in = nc.dram_tensor("qaugTa", [102, M], F32, kind="ExternalInput").ap()
    qnr_in = nc.dram_tensor("qnr", [M, D], BF16, kind="ExternalInput").ap()
    id_in = nc.dram_tensor("ident", [P, P], F32, kind="ExternalInput").ap()
    g_out = nc.dram_tensor("g", [R, 4 * D], F32, kind="ExternalOutput").ap()

    with tile.TileContext(nc) as tc:
        with ExitStack() as ex:
            build_body(nc, tc, ex, ctx_in, ctxTa_in, qaugTa_in, qnr_in,
                       id_in, g_out)
    return nc


def build_body(nc, tc, ex, ctx_in, ctxTa_in, qaugTa_in, qnr_in, id_in, g_out):
    sing = ex.enter_context(tc.tile_pool(name="sing", bufs=1))
    ptt_pool = ex.enter_context(tc.tile_pool(name="ptt", bufs=4))
    m1_pool = ex.enter_context(tc.tile_pool(name="m1", bufs=2))
    uat_pool = ex.enter_context(tc.tile_pool(name="uat", bufs=2))
    g12_pool = ex.enter_context(tc.tile_pool(name="g12", bufs=3))
    # PSUM: stp 2x[128,1536]f32 (6 banks) + UA [101,512] (1) + tiny (1) = 8
    stp = ex.enter_context(tc.tile_pool(name="stp", bufs=2, space="PSUM"))
    uap = ex.enter_context(tc.tile_pool(name="uap", bufs=1, space="PSUM"))
    tp = ex.enter_context(tc.tile_pool(name="tp", bufs=1, space="PSUM"))
    dram = ex.enter_context(tc.tile_pool(name="dram", bufs=1, space="DRAM"))

    # ---- persistent SBUF ----
    caugT = sing.tile([102, R], F32R)  # 0..99 ctxT | 100 ones | 101 c.w1 - B
    qaugT = sing.tile([102, M], F32R)  # 0..99 qT*w3 | 100 q.w2 | 101 ones
    stg_c = sing.tile([102, R], F32)
    stg_q = sing.tile([102, M], F32)
    qaugN = sing.tile([P, QC, 104], BF16)  # q natural chunks + ones col @100
    ctxn = sing.tile([P, NCH, 104], F32)   # ctx natural chunks + ones col @100
    tid = sing.tile([P, P], F32)
    ystore = sing.tile([P, NCH], F32)      # y = exp(rowmax - B), natural
    uan = sing.tile([P, NCH, 104], F32)    # U_A unnorm natural + Z col @100
    rzs = sing.tile([P, NCH], F32)         # 1/Z per chunk
    ones1 = sing.tile([1, P], F32)
    ones81 = sing.tile([N_CORES, 1], F32)
    hlacc = sing.tile([101, 1], F32)
    hltmp = sing.tile([101, 1], F32)
    hlrow = sing.tile([1, 101], F32)
    hB = sing.tile([P, D], F32)
    g3big = sing.tile([P, NCH, D], F32)
    dummy = sing.tile([1, 1], F32)

    cc_in = dram.tile([1, 102], F32)
    cc_out = dram.tile([N_CORES, 102], F32)
    cc_w_in = dram.tile([1, 8], F32)
    cc_w_out = dram.tile([N_CORES, 8], F32)

    # ---- input loads (critical first: qaugT piece 0 + caugT tile 0) ----
    # f32r matmul inputs must be produced by a rounding instruction, so DMA
    # lands in f32 staging and ACT/DVE copy-round into the f32r tiles. The
    # two pieces that gate the pipeline start go on ACT, the rest on DVE.
    # Bulk, non-gating loads trigger from the idle Pool queue.
    nc.sync.dma_start(out=stg_c[:, 0:TW], in_=ctxTa_in[:, 0:TW])
    nc.sync.dma_start(out=stg_q[:, 0:384], in_=qaugTa_in[:, 0:384])
    nc.sync.dma_start(out=stg_q[:, 384:1024], in_=qaugTa_in[:, 384:1024])
    nc.vector.memset(dummy[:], 0.0)
    # preload the exp table set early (hidden behind input DMAs); keep the
    # ACT queue free of DMA triggers so the gating copies run ASAP
    nc.scalar.activation(dummy[:], dummy[:], EXP)
    nc.scalar.activation(caugT[:, 0:TW], stg_c[:, 0:TW], COPY)
    nc.scalar.activation(qaugT[:, 0:384], stg_q[:, 0:384], COPY)
    # q640 copy goes on DVE: on ACT it would queue ahead of exp(0) and gate
    # the whole pipeline on its (late) DMA
    nc.vector.tensor_copy(qaugT[:, 384:1024], stg_q[:, 384:1024])
    # gating loads first: qaugT pieces + caugT tiles feed the S pipeline
    # directly; bulk non-gating loads (qaugN/tid/ctxn/g0) are held behind
    # the last staging copy so their transfers don't hog the DMA engines.
    p3copy = None
    for piece in range(1, 4):
        nc.sync.dma_start(out=stg_q[:, piece * 1024:(piece + 1) * 1024],
                          in_=qaugTa_in[:, piece * 1024:(piece + 1) * 1024])
        p3copy = nc.vector.tensor_copy(
            qaugT[:, piece * 1024:(piece + 1) * 1024],
            stg_q[:, piece * 1024:(piece + 1) * 1024])
    for t in range(1, NT):
        nc.sync.dma_start(out=stg_c[:, t * TW:(t + 1) * TW],
                          in_=ctxTa_in[:, t * TW:(t + 1) * TW])
        nc.vector.tensor_copy(caugT[:, t * TW:(t + 1) * TW],
                              stg_c[:, t * TW:(t + 1) * TW])
    from concourse.tile_rust import add_dep_helper as _adh
    nc.vector.memset(qaugN[:, :, 100:104], 1.0)
    nc.gpsimd.dma_start(
        out=qaugN[:, :, 0:D],
        in_=qnr_in.rearrange("(c p) d -> p c d", p=P))
    d_tid = nc.gpsimd.dma_start(out=tid[:], in_=id_in[:])
    nc.vector.memset(ctxn[:, :, 100:104], 1.0)
    d_ctxn = nc.gpsimd.dma_start(
        out=ctxn[:, :, 0:D],
        in_=ctx_in.rearrange("(c p) d -> p c d", p=P))
    nc.vector.memset(ones1[:], 1.0)
    nc.vector.memset(ones81[:], 1.0)

    # G cols 0:100 = context verbatim (DRAM->DRAM); least urgent load
    d_g0 = nc.gpsimd.dma_start(out=g_out[:, 0:D], in_=ctx_in[:])
    for d in (d_tid, d_ctxn, d_g0):
        _adh(d.ins, p3copy.ins, sync=True, reason="bulk loads after staging")

    # Warm-up AllGather doubling as a cross-core barrier: absorbs NEFF start
    # skew and warms the CC rings so the real end-of-loop collective only
    # pays its intrinsic latency.
    nc.gpsimd.collective_compute(
        "AllGather", mybir.AluOpType.bypass,
        replica_groups=[list(range(N_CORES))],
        ins=[cc_w_in.opt()], outs=[cc_w_out.opt()])

    # q-chunk groups: 10x3 + 1x2
    groups = [list(range(g, min(g + GRP, QC))) for g in range(0, QC, GRP)]

    def q2c_dve(t, m1):
        """Fold the 3-slot running max down to y-per-ctx-col (DVE only)."""
        tmpm = m1_pool.tile([P, TW], BF16, tag="tmpm")
        nc.vector.tensor_max(tmpm[:], m1[:, 0:TW], m1[:, TW:2 * TW])
        yt = m1_pool.tile([P, TW], F32, tag="yt")
        nc.vector.tensor_max(yt[:], tmpm[:], m1[:, 2 * TW:3 * TW])
        return yt

    def ua_evict(t, uaps):
        uat = uat_pool.tile([101, TW], F32, tag="uat")
        nc.vector.tensor_copy(uat[:], uaps[:])
        return uat

    def q2c_pe_a(t, yt):
        """y transpose to natural layout (PE) + column-max reduces."""
        yps4 = tp.tile([P, CPT * P], F32, tag="tiny", name=f"yps4_{t}")
        for ci in range(CPT):
            nc.tensor.transpose(yps4[:, ci * P:(ci + 1) * P],
                                yt[:, ci * P:(ci + 1) * P], tid[:])
        for ci in range(CPT):
            cc = t * CPT + ci
            nc.vector.reduce_max(ystore[:, cc:cc + 1],
                                 yps4[:, ci * P:(ci + 1) * P],
                                 axis=mybir.AxisListType.X)

    def q2c_pe_b(t):
        """hl partial accumulation into hlacc (+ row form for the last)."""
        hlp = tp.tile([101, 1], F32, tag="tiny", name=f"hlp_{t}")
        for ci in range(CPT):
            cc = t * CPT + ci
            nc.tensor.matmul(hlp[:], ctxn[:, cc, 0:101], ystore[:, cc:cc + 1],
                             start=(ci == 0), stop=(ci == CPT - 1))
        if t == 0:
            nc.vector.tensor_copy(hlacc[:], hlp[:])
        else:
            nc.vector.tensor_copy(hltmp[:], hlp[:])
            nc.vector.tensor_add(hlacc[:], hlacc[:], hltmp[:])
        if t == NT - 1:
            # row-form hl so the collective-input DMA is one descriptor
            hlrps = tp.tile([1, 101], F32, tag="tiny", name="hlr")
            nc.tensor.transpose(hlrps[:], hlacc[:], tid[0:101, 0:101])
            nc.vector.tensor_copy(hlrow[:], hlrps[:])

    def ua_norm2(t, uat, half):
        """U_A normalize + G cols 100:300 for 2 of the 4 chunks."""
        first_pool = None
        for ci in range(2 * half, 2 * half + 2):
            cc = t * CPT + ci
            uanps = tp.tile([P, 101], F32, tag="tiny", name=f"uanps_{cc}")
            nc.tensor.transpose(uanps[:], uat[:, ci * P:(ci + 1) * P],
                                tid[0:101, 0:101])
            nc.vector.tensor_copy(uan[:, cc, 0:101], uanps[:])
            nc.vector.reciprocal(rzs[:, cc:cc + 1], uan[:, cc, 100:101])
            g12 = g12_pool.tile([P, 2 * D], F32, tag="g12")
            nc.vector.tensor_scalar_mul(g12[:, 0:D], uan[:, cc, 0:D],
                                        rzs[:, cc:cc + 1])
            g2op = nc.gpsimd.tensor_mul(g12[:, D:2 * D], ctxn[:, cc, 0:D],
                                        g12[:, 0:D])
            if first_pool is None:
                first_pool = g2op
            last = nc.sync.dma_start(out=g_out[cc * P:(cc + 1) * P, D:3 * D],
                                     in_=g12[:])
        return last, first_pool

    # ---- flat cross-tile software pipeline: UA lags S/exp by one group
    # globally (so the PE never blocks on exp, even across tile boundaries),
    # and the previous tile's tail work is spread one slice per group. ----
    slots = [(t, gi, chunks) for t in range(NT)
             for gi, chunks in enumerate(groups)]
    NG = len(groups)
    state = {}   # per-tile m1/uaps/ptts
    prev_tail = None
    t3_last = t3_pool = None

    def emit_ua(k):
        pt, pgi, pchunks = slots[k]
        pptt = state[pt]["ptts"][pgi]
        for j, qc in enumerate(pchunks):
            nc.tensor.matmul(
                state[pt]["uaps"][:], qaugN[:, qc, 0:101],
                pptt[:, j * TW:(j + 1) * TW],
                start=(qc == 0), stop=(qc == QC - 1))

    for k, (t, gi, chunks) in enumerate(slots):
        ctxsl = caugT[:, t * TW:(t + 1) * TW]
        w = len(chunks) * TW
        sp = stp.tile([P, GRP * TW], F32, tag="sp")
        for j, qc in enumerate(chunks):
            nc.tensor.matmul(
                sp[:, j * TW:(j + 1) * TW],
                qaugT[:, qc * P:(qc + 1) * P],
                ctxsl, start=True, stop=True)
        if k > 0:
            emit_ua(k - 1)
        if gi == 0 and t > 0:
            # previous tile's uaps evict + y folds right at the boundary
            pt = t - 1
            uat = ua_evict(pt, state[pt]["uaps"])
            yt = q2c_dve(pt, state[pt]["m1"])
            prev_tail = (pt, yt, uat)
        if gi == 0:
            state[t] = {
                "m1": m1_pool.tile([P, GRP * TW], BF16, tag="m1",
                                   name=f"m1_{t}"),
                "uaps": uap.tile([101, TW], F32, tag="ua",
                                 name=f"uaps_{t}"),
                "ptts": [],
            }
        ptt = ptt_pool.tile([P, GRP * TW], BF16, tag="ptt",
                            name=f"ptt_{t}_{gi}")
        state[t]["ptts"].append(ptt)
        nc.scalar.activation(ptt[:, 0:w], sp[:, 0:w], EXP)
        m1 = state[t]["m1"]
        if gi == 0:
            nc.vector.tensor_copy(m1[:, 0:w], ptt[:, 0:w])
        else:
            nc.vector.tensor_max(m1[:, 0:w], m1[:, 0:w], ptt[:, 0:w])
        if prev_tail is not None:
            pt, yt, uat = prev_tail
            if gi == 2:
                q2c_pe_a(pt, yt)
            elif gi == 3:
                q2c_pe_b(pt)
            elif gi == 5:
                ua_norm2(pt, uat, 0)
            elif gi == 7:
                ua_norm2(pt, uat, 1)

    emit_ua(len(slots) - 1)
    # last tile tail: folds first so the Q2C/collective chain starts ASAP
    t = NT - 1
    yt = q2c_dve(t, state[t]["m1"])
    uat3 = ua_evict(t, state[t]["uaps"])
    q2c_pe_a(t, yt)
    q2c_pe_b(t)

    # ---- Q2C partials + AllGather (emitted before tile-3's G output so the
    # collective triggers the moment hlacc is ready) ----
    nc.gpsimd.dma_start(out=cc_in[0:1, 0:101], in_=hlrow[:])
    cc_inst = nc.gpsimd.collective_compute(
        "AllGather", mybir.AluOpType.bypass,
        replica_groups=[list(range(N_CORES))],
        ins=[cc_in.opt()], outs=[cc_out.opt()])

    _, t3_pool = ua_norm2(NT - 1, uat3, 0)
    t3_last, _ = ua_norm2(NT - 1, uat3, 1)
    # ordering-only edge: keep the AllGather trigger ahead of tile-3's Pool
    # work (the tile scheduler orders by deps, not emission order)
    _adh(t3_pool.ins, cc_inst.ins, sync=False,
         reason="collective before tile-3 pool muls")

    # ---- combine after AllGather ----
    agm = sing.tile([N_CORES, 102], F32)
    d1 = nc.sync.dma_start(out=agm[:], in_=cc_out[:])
    # keep the combine's load from stalling engines mid-loop
    _adh(d1.ins, t3_last.ins, sync=True, reason="combine after tile3")
    hsps = tp.tile([1, 102], F32, tag="tiny", name="hsps")
    nc.tensor.matmul(hsps[:], ones81[:], agm[:], start=True, stop=True)
    hsum = sing.tile([1, 102], F32)
    nc.scalar.activation(hsum[:], hsps[:], COPY)
    rzh = sing.tile([1, 1], F32)
    nc.vector.reciprocal(rzh[:], hsum[:, 100:101])
    hrow = sing.tile([1, D], F32)
    nc.vector.tensor_scalar_mul(hrow[:], hsum[:, 0:D], rzh[:])
    hbps = tp.tile([P, D], F32, tag="tiny", name="hbps")
    nc.tensor.matmul(hbps[:], ones1[:], hrow[:], start=True, stop=True)
    nc.scalar.activation(hB[:], hbps[:], COPY)
    # 8 half-tile stores alternating Sync/Pool queues so the 0.82MB block-4
    # output lands on several DMA engines instead of serializing on one
    # queue (the last transfer gates the kernel's drain)
    for half in range(2 * NT):
        for ci in range(2):
            cc = 2 * half + ci
            nc.vector.tensor_mul(g3big[:, cc, :], ctxn[:, cc, 0:D], hB[:])
        eng = nc.sync if half % 2 == 0 else nc.gpsimd
        eng.dma_start(
            out=g_out[half * 256:(half + 1) * 256, 3 * D:4 * D]
            .rearrange("(c p) d -> p c d", p=P),
            in_=g3big[:, 2 * half:2 * half + 2, :])


_nc_cache = None


def _get_nc():
    global _nc_cache
    if _nc_cache is None:
        _nc_cache = build_bass()
        split_multi_waits(_nc_cache)
    return _nc_cache


def _prep_inputs(inputs):
    context = np.ascontiguousarray(inputs["context"], dtype=np.float32)
    question = np.ascontiguousarray(inputs["question"], dtype=np.float32)
    kern = np.ascontiguousarray(inputs["kernel"], dtype=np.float32)
    w1, w2, w3 = kern[:D], kern[D:2 * D], kern[2 * D:]

    # Global stability constant B. exp(S-B) must neither overflow fp32 on
    # the global max nor underflow whole rows to 0 in bf16 (Z=0 -> NaN), so
    # B must sit in [max(S)-80, min_i(rowmax_i)+85]. Row maxes are spread by
    # the per-row c.w1 offset, so estimate them with a q-subsample GEMM and
    # center B in the window.
    cw1 = context @ w1
    q2 = question @ w2
    nq = question.shape[0]
    qsub = question[::16]
    tmax = ((context * w3[None, :]) @ qsub.T + (qsub @ w2)[None, :]).max(axis=1)
    rm = cw1 + tmax
    B = float(0.5 * (rm.max() + rm.min() + 20.0))

    qaugTa = np.empty((102, nq), np.float32)
    qaugTa[0:D] = (question * w3[None, :]).T
    qaugTa[D] = q2
    qaugTa[D + 1] = 1.0
    qaugTa = np.ascontiguousarray(qaugTa)
    import ml_dtypes
    qbf = np.ascontiguousarray(question.astype(ml_dtypes.bfloat16))
    ident = np.eye(P, dtype=np.float32)

    in_maps = []
    for k in range(N_CORES):
        cshard = np.ascontiguousarray(context[k * R:(k + 1) * R])
        ctxTa = np.empty((102, R), np.float32)
        ctxTa[0:D] = cshard.T
        ctxTa[D] = 1.0
        ctxTa[D + 1] = cw1[k * R:(k + 1) * R] - B
        in_maps.append({
            "ctx": cshard,
            "ctxTa": np.ascontiguousarray(ctxTa),
            "qaugTa": qaugTa,
            "qnr": qbf,
            "ident": ident,
        })
    return in_maps


def kernel(**inputs):
    from concourse.bass_utils import run_bass_kernel_spmd

    in_maps = _prep_inputs(inputs)
    res = run_bass_kernel_spmd(_get_nc(), in_maps,
                               core_ids=list(range(N_CORES)))
    return np.concatenate([res.results[k]["g"] for k in range(N_CORES)],
                          axis=0)


def kernel_traced(**inputs):
    """Like kernel() but also returns HW exec time in ns (NTFF profile)."""
    from concourse.bass_utils import run_bass_kernel_spmd

    kernel(**inputs)  # warm compile via cached nc
    in_maps = _prep_inputs(inputs)
    res = run_bass_kernel_spmd(_get_nc(), in_maps,
                               core_ids=list(range(N_CORES)), trace=True)
    out = np.concatenate([res.results[k]["g"] for k in range(N_CORES)],
                         axis=0)
    return out, res


# revision 66
# speedup vs baseline: 1.1119x; 1.0090x over previous
"""BiAttention (BiDAF) Trainium2 Bass kernel — 8 NeuronCores, sequence-
parallel over the context axis.

kernel(context [16384,100] f32, question [4096,100] f32, kernel [300] f32)
  -> G [16384, 400] f32  (concat: ctx | U_A | ctx*U_A | ctx*H_A)

Single-S-pass scheme. A host-computed global stability constant B is folded
into the c.w1 bias row, so each PSUM S-chunk already holds S-B and ACT exps
it straight into bf16 ptt (the exact row-max pass of the two-pass scheme is
gone). U_A/Z accumulate on the PE from bf16 ptt at 1 cyc/row. The exact Q2C
row-maxes come from a bf16 running elementwise max over ptt (DVE 2x mode)
folded + PE-transposed per ctx tile: y = exp(m - B), so the Q2C softmax is
just y/sum(y) — no exp/log — and with a global B the cross-core combine
after the 102-float AllGather is a plain 8-row sum.
"""
import sys

sys.path.insert(0, "/opt/trn_rl_repo")
from contextlib import ExitStack

import numpy as np

import concourse.bass as bass
import concourse.tile as tile
from concourse import mybir


def split_multi_waits(nc):
    """This walrus build rejects instructions with >1 sync wait. Hoist extra
    waits onto single-wait EventSemaphore nops on the same engine (engines
    execute in order, so N sequential single waits == one N-way wait)."""
    n_split = 0
    counter = [0]

    def make_nop(engine, wait):
        counter[0] += 1
        inst = mybir.InstEventSemaphore(
            name=f"I-waitsplit-{counter[0]}", ins=[], outs=[])
        inst.engine = engine
        inst.sync_info = mybir.SyncInfo(on_wait=[wait], on_update=[])
        return inst

    for f in nc.m.functions:
        for blk in f.blocks:
            changed = False
            new_insts = []
            for inst in blk.instructions:
                si = inst.sync_info
                if si is not None and si.on_wait and len(si.on_wait) > 1:
                    waits = list(si.on_wait)
                    for w in waits[:-1]:
                        new_insts.append(make_nop(inst.engine, w))
                    si.on_wait = [waits[-1]]
                    n_split += 1
                    changed = True
                new_insts.append(inst)
            if changed:
                blk.instructions[:] = new_insts
    return n_split


F32 = mybir.dt.float32
F32R = mybir.dt.float32r
BF16 = mybir.dt.bfloat16
EXP = mybir.ActivationFunctionType.Exp
COPY = mybir.ActivationFunctionType.Copy

N_CORES = 8
D = 100
R = 2048          # ctx rows per core
M = 4096          # question rows
P = 128           # partitions
NCH = R // P      # 16 ctx chunks (natural layout)
QC = M // P       # 32 q chunks
TW = 512          # ctx tile width
NT = R // TW      # 4 ctx tiles
CPT = TW // P     # 4 ctx chunks per tile
GRP = 3           # q-chunks per exp group (3 PSUM banks)


def build_bass():
    nc = bass.Bass("TRN2", target_bir_lowering=False, debug=False,
                   num_devices=N_CORES)
    ctx_in = nc.dram_tensor("ctx", [R, D], F32, kind="ExternalInput").ap()
    ctxTa_in = nc.dram_tensor("ctxTa", [102, R], F32, kind="ExternalInput").ap()
    qaugTa_in = nc.dram_tensor("qaugTa", [102, M], F32, kind="ExternalInput").ap()
    qnr_in = nc.dram_tensor("qnr", [M, D], BF16, kind="ExternalInput").ap()
    id_in = nc.dram_tensor("ident", [P, P], F32, kind="ExternalInput").ap()
    g_out = nc.dram_tensor("g", [R, 4 * D], F32, kind="ExternalOutput").ap()

    with tile.TileContext(nc) as tc:
        with ExitStack() as ex:
            build_body(nc, tc, ex, ctx_in, ctxTa_in, qaugTa_in, qnr_in,
                       id_in, g_out)
    return nc


def build_body(nc, tc, ex, ctx_in, ctxTa_in, qaugTa_in, qnr_in, id_in, g_out):
    sing = ex.enter_context(tc.tile_pool(name="sing", bufs=1))
    ptt_pool = ex.enter_context(tc.tile_pool(name="ptt", bufs=4))
    m1_pool = ex.enter_context(tc.tile_pool(name="m1", bufs=2))
    uat_pool = ex.enter_context(tc.tile_pool(name="uat", bufs=2))
    g12_pool = ex.enter_context(tc.tile_pool(name="g12", bufs=3))
    # PSUM: stp 2x[128,1536]f32 (6 banks) + UA [101,512] (1) + tiny (1) = 8
    stp = ex.enter_context(tc.tile_pool(name="stp", bufs=2, space="PSUM"))
    uap = ex.enter_context(tc.tile_pool(name="uap", bufs=1, space="PSUM"))
    tp = ex.enter_context(tc.tile_pool(name="tp", bufs=1, space="PSUM"))
    dram = ex.enter_context(tc.tile_pool(name="dram", bufs=1, space="DRAM"))

    # ---- persistent SBUF ----
    caugT = sing.tile([102, R], F32R)  # 0..99 ctxT | 100 ones | 101 c.w1 - B
    qaugT = sing.tile([102, M], F32R)  # 0..99 qT*w3 | 100 q.w2 | 101 ones
    stg_c = sing.tile([102, R], F32)
    stg_q = sing.tile([102, M], F32)
    qaugN = sing.tile([P, QC, 104], BF16)  # q natural chunks + ones col @100
    ctxn = sing.tile([P, NCH, 104], F32)   # ctx natural chunks + ones col @100
    tid = sing.tile([P, P], F32)
    ystore = sing.tile([P, NCH], F32)      # y = exp(rowmax - B), natural
    uan = sing.tile([P, NCH, 104], F32)    # U_A unnorm natural + Z col @100
    rzs = sing.tile([P, NCH], F32)         # 1/Z per chunk
    ones1 = sing.tile([1, P], F32)
    ones81 = sing.tile([N_CORES, 1], F32)
    hlacc = sing.tile([101, 1], F32)
    hltmp = sing.tile([101, 1], F32)
    hlrow = sing.tile([1, 101], F32)
    hB = sing.tile([P, D], F32)
    g3big = sing.tile([P, NCH, D], F32)
    dummy = sing.tile([1, 1], F32)

    cc_in = dram.tile([1, 102], F32)
    cc_out = dram.tile([N_CORES, 102], F32)
    cc_w_in = dram.tile([1, 8], F32)
    cc_w_out = dram.tile([N_CORES, 8], F32)

    # ---- input loads (critical first: qaugT piece 0 + caugT tile 0) ----
    # f32r matmul inputs must be produced by a rounding instruction, so DMA
    # lands in f32 staging and ACT/DVE copy-round into the f32r tiles. The
    # two pieces that gate the pipeline start go on ACT, the rest on DVE.
    # Bulk, non-gating loads trigger from the idle Pool queue.
    nc.sync.dma_start(out=stg_c[:, 0:TW], in_=ctxTa_in[:, 0:TW])
    nc.sync.dma_start(out=stg_q[:, 0:384], in_=qaugTa_in[:, 0:384])
    nc.sync.dma_start(out=stg_q[:, 384:1024], in_=qaugTa_in[:, 384:1024])
    nc.vector.memset(dummy[:], 0.0)
    # preload the exp table set early (hidden behind input DMAs); keep the
    # ACT queue free of DMA triggers so the gating copies run ASAP
    nc.scalar.activation(dummy[:], dummy[:], EXP)
    nc.scalar.activation(caugT[:, 0:TW], stg_c[:, 0:TW], COPY)
    nc.scalar.activation(qaugT[:, 0:384], stg_q[:, 0:384], COPY)
    # q640 copy goes on DVE: on ACT it would queue ahead of exp(0) and gate
    # the whole pipeline on its (late) DMA
    nc.vector.tensor_copy(qaugT[:, 384:1024], stg_q[:, 384:1024])
    # gating loads first: qaugT pieces + caugT tiles feed the S pipeline
    # directly; bulk non-gating loads (qaugN/tid/ctxn/g0) are held behind
    # the last staging copy so their transfers don't hog the DMA engines.
    p3copy = None
    for piece in range(1, 4):
        nc.sync.dma_start(out=stg_q[:, piece * 1024:(piece + 1) * 1024],
                          in_=qaugTa_in[:, piece * 1024:(piece + 1) * 1024])
        p3copy = nc.vector.tensor_copy(
            qaugT[:, piece * 1024:(piece + 1) * 1024],
            stg_q[:, piece * 1024:(piece + 1) * 1024])
    for t in range(1, NT):
        nc.sync.dma_start(out=stg_c[:, t * TW:(t + 1) * TW],
                          in_=ctxTa_in[:, t * TW:(t + 1) * TW])
        nc.vector.tensor_copy(caugT[:, t * TW:(t + 1) * TW],
                              stg_c[:, t * TW:(t + 1) * TW])
    from concourse.tile_rust import add_dep_helper as _adh
    nc.vector.memset(qaugN[:, :, 100:104], 1.0)
    nc.gpsimd.dma_start(
        out=qaugN[:, :, 0:D],
        in_=qnr_in.rearrange("(c p) d -> p c d", p=P))
    d_tid = nc.gpsimd.dma_start(out=tid[:], in_=id_in[:])
    nc.vector.memset(ctxn[:, :, 100:104], 1.0)
    d_ctxn = nc.gpsimd.dma_start(
        out=ctxn[:, :, 0:D],
        in_=ctx_in.rearrange("(c p) d -> p c d", p=P))
    nc.vector.memset(ones1[:], 1.0)
    nc.vector.memset(ones81[:], 1.0)

    # G cols 0:100 = context verbatim (DRAM->DRAM); least urgent load
    d_g0 = nc.gpsimd.dma_start(out=g_out[:, 0:D], in_=ctx_in[:])
    for d in (d_tid, d_ctxn, d_g0):
        _adh(d.ins, p3copy.ins, sync=True, reason="bulk loads after staging")

    # Warm-up AllGather doubling as a cross-core barrier: absorbs NEFF start
    # skew and warms the CC rings so the real end-of-loop collective only
    # pays its intrinsic latency.
    nc.gpsimd.collective_compute(
        "AllGather", mybir.AluOpType.bypass,
        replica_groups=[list(range(N_CORES))],
        ins=[cc_w_in.opt()], outs=[cc_w_out.opt()])

    # q-chunk groups: 10x3 + 1x2
    groups = [list(range(g, min(g + GRP, QC))) for g in range(0, QC, GRP)]

    def q2c_dve(t, m1):
        """Fold the 3-slot running max down to y-per-ctx-col (DVE only)."""
        tmpm = m1_pool.tile([P, TW], BF16, tag="tmpm")
        nc.vector.tensor_max(tmpm[:], m1[:, 0:TW], m1[:, TW:2 * TW])
        yt = m1_pool.tile([P, TW], F32, tag="yt")
        nc.vector.tensor_max(yt[:], tmpm[:], m1[:, 2 * TW:3 * TW])
        return yt

    def ua_evict(t, uaps):
        uat = uat_pool.tile([101, TW], F32, tag="uat")
        nc.vector.tensor_copy(uat[:], uaps[:])
        return uat

    def q2c_pe_a(t, yt):
        """y transpose to natural layout (PE) + column-max reduces."""
        yps4 = tp.tile([P, CPT * P], F32, tag="tiny", name=f"yps4_{t}")
        for ci in range(CPT):
            nc.tensor.transpose(yps4[:, ci * P:(ci + 1) * P],
                                yt[:, ci * P:(ci + 1) * P], tid[:])
        for ci in range(CPT):
            cc = t * CPT + ci
            nc.vector.reduce_max(ystore[:, cc:cc + 1],
                                 yps4[:, ci * P:(ci + 1) * P],
                                 axis=mybir.AxisListType.X)

    def q2c_pe_b(t):
        """hl partial accumulation into hlacc (+ row form for the last)."""
        hlp = tp.tile([101, 1], F32, tag="tiny", name=f"hlp_{t}")
        for ci in range(CPT):
            cc = t * CPT + ci
            nc.tensor.matmul(hlp[:], ctxn[:, cc, 0:101], ystore[:, cc:cc + 1],
                             start=(ci == 0), stop=(ci == CPT - 1))
        if t == 0:
            nc.vector.tensor_copy(hlacc[:], hlp[:])
        else:
            nc.vector.tensor_copy(hltmp[:], hlp[:])
            nc.vector.tensor_add(hlacc[:], hlacc[:], hltmp[:])
        if t == NT - 1:
            # row-form hl so the collective-input DMA is one descriptor
            hlrps = tp.tile([1, 101], F32, tag="tiny", name="hlr")
            nc.tensor.transpose(hlrps[:], hlacc[:], tid[0:101, 0:101])
            nc.vector.tensor_copy(hlrow[:], hlrps[:])

    def ua_norm2(t, uat, half):
        """U_A normalize + G cols 100:300 for 2 of the 4 chunks."""
        first_pool = None
        for ci in range(2 * half, 2 * half + 2):
            cc = t * CPT + ci
            uanps = tp.tile([P, 101], F32, tag="tiny", name=f"uanps_{cc}")
            nc.tensor.transpose(uanps[:], uat[:, ci * P:(ci + 1) * P],
                                tid[0:101, 0:101])
            nc.vector.tensor_copy(uan[:, cc, 0:101], uanps[:])
            nc.vector.reciprocal(rzs[:, cc:cc + 1], uan[:, cc, 100:101])
            g12 = g12_pool.tile([P, 2 * D], F32, tag="g12")
            nc.vector.tensor_scalar_mul(g12[:, 0:D], uan[:, cc, 0:D],
                                        rzs[:, cc:cc + 1])
            g2op = nc.gpsimd.tensor_mul(g12[:, D:2 * D], ctxn[:, cc, 0:D],
                                        g12[:, 0:D])
            if first_pool is None:
                first_pool = g2op
            last = nc.sync.dma_start(out=g_out[cc * P:(cc + 1) * P, D:3 * D],
                                     in_=g12[:])
        return last, first_pool

    # ---- flat cross-tile software pipeline: UA lags S/exp by one group
    # globally (so the PE never blocks on exp, even across tile boundaries),
    # and the previous tile's tail work is spread one slice per group. ----
    slots = [(t, gi, chunks) for t in range(NT)
             for gi, chunks in enumerate(groups)]
    NG = len(groups)
    state = {}   # per-tile m1/uaps/ptts
    prev_tail = None
    t3_last = t3_pool = None

    def emit_ua(k):
        pt, pgi, pchunks = slots[k]
        pptt = state[pt]["ptts"][pgi]
        for j, qc in enumerate(pchunks):
            nc.tensor.matmul(
                state[pt]["uaps"][:], qaugN[:, qc, 0:101],
                pptt[:, j * TW:(j + 1) * TW],
                start=(qc == 0), stop=(qc == QC - 1))

    for k, (t, gi, chunks) in enumerate(slots):
        ctxsl = caugT[:, t * TW:(t + 1) * TW]
        w = len(chunks) * TW
        sp = stp.tile([P, GRP * TW], F32, tag="sp")
        for j, qc in enumerate(chunks):
            nc.tensor.matmul(
                sp[:, j * TW:(j + 1) * TW],
                qaugT[:, qc * P:(qc + 1) * P],
                ctxsl, start=True, stop=True)
        if k > 0:
            emit_ua(k - 1)
        if gi == 0 and t > 0:
            # previous tile's uaps evict + y folds right at the boundary
            pt = t - 1
            uat = ua_evict(pt, state[pt]["uaps"])
            yt = q2c_dve(pt, state[pt]["m1"])
            prev_tail = (pt, yt, uat)
        if gi == 0:
            state[t] = {
                "m1": m1_pool.tile([P, GRP * TW], BF16, tag="m1",
                                   name=f"m1_{t}"),
                "uaps": uap.tile([101, TW], F32, tag="ua",
                                 name=f"uaps_{t}"),
                "ptts": [],
            }
        ptt = ptt_pool.tile([P, GRP * TW], BF16, tag="ptt",
                            name=f"ptt_{t}_{gi}")
        state[t]["ptts"].append(ptt)
        nc.scalar.activation(ptt[:, 0:w], sp[:, 0:w], EXP)
        m1 = state[t]["m1"]
        if gi == 0:
            nc.vector.tensor_copy(m1[:, 0:w], ptt[:, 0:w])
        else:
            nc.vector.tensor_max(m1[:, 0:w], m1[:, 0:w], ptt[:, 0:w])
        if prev_tail is not None:
            pt, yt, uat = prev_tail
            if gi == 2:
                q2c_pe_a(pt, yt)
            elif gi == 3:
                q2c_pe_b(pt)
            elif gi == 5:
                ua_norm2(pt, uat, 0)
            elif gi == 7:
                ua_norm2(pt, uat, 1)

    emit_ua(len(slots) - 1)
    # last tile tail: folds first so the Q2C/collective chain starts ASAP
    t = NT - 1
    yt = q2c_dve(t, state[t]["m1"])
    uat3 = ua_evict(t, state[t]["uaps"])
    q2c_pe_a(t, yt)
    q2c_pe_b(t)

    # ---- Q2C partials + AllGather (emitted before tile-3's G output so the
    # collective triggers the moment hlacc is ready) ----
    nc.gpsimd.dma_start(out=cc_in[0:1, 0:101], in_=hlrow[:])
    cc_inst = nc.gpsimd.collective_compute(
        "AllGather", mybir.AluOpType.bypass,
        replica_groups=[list(range(N_CORES))],
        ins=[cc_in.opt()], outs=[cc_out.opt()])

    _, t3_pool = ua_norm2(NT - 1, uat3, 0)
    t3_last, _ = ua_norm2(NT - 1, uat3, 1)
    # ordering-only edge: keep the AllGather trigger ahead of tile-3's Pool
    # work (the tile scheduler orders by deps, not emission order)
    _adh(t3_pool.ins, cc_inst.ins, sync=False,
         reason="collective before tile-3 pool muls")

    # ---- combine after AllGather ----
    agm = sing.tile([N_CORES, 102], F32)
    d1 = nc.sync.dma_start(out=agm[:], in_=cc_out[:])
    # keep the combine's load from stalling engines mid-loop
    _adh(d1.ins, t3_last.ins, sync=True, reason="combine after tile3")
    hsps = tp.tile([1, 102], F32, tag="tiny", name="hsps")
    nc.tensor.matmul(hsps[:], ones81[:], agm[:], start=True, stop=True)
    hsum = sing.tile([1, 102], F32)
    nc.scalar.activation(hsum[:], hsps[:], COPY)
    rzh = sing.tile([1, 1], F32)
    nc.vector.reciprocal(rzh[:], hsum[:, 100:101])
    hrow = sing.tile([1, D], F32)
    nc.vector.tensor_scalar_mul(hrow[:], hsum[:, 0:D], rzh[:])
    hbps = tp.tile([P, D], F32, tag="tiny", name="hbps")
    nc.tensor.matmul(hbps[:], ones1[:], hrow[:], start=True, stop=True)
    nc.scalar.activation(hB[:], hbps[:], COPY)
    for t in range(NT):
        for ci in range(CPT):
            cc = t * CPT + ci
            nc.vector.tensor_mul(g3big[:, cc, :], ctxn[:, cc, 0:D], hB[:])
        nc.sync.dma_start(
            out=g_out[t * TW:(t + 1) * TW, 3 * D:4 * D]
            .rearrange("(c p) d -> p c d", p=P),
            in_=g3big[:, t * CPT:(t + 1) * CPT, :])


_nc_cache = None


def _get_nc():
    global _nc_cache
    if _nc_cache is None:
        _nc_cache = build_bass()
        split_multi_waits(_nc_cache)
    return _nc_cache


def _prep_inputs(inputs):
    context = np.ascontiguousarray(inputs["context"], dtype=np.float32)
    question = np.ascontiguousarray(inputs["question"], dtype=np.float32)
    kern = np.ascontiguousarray(inputs["kernel"], dtype=np.float32)
    w1, w2, w3 = kern[:D], kern[D:2 * D], kern[2 * D:]

    # Global stability constant B. exp(S-B) must neither overflow fp32 on
    # the global max nor underflow whole rows to 0 in bf16 (Z=0 -> NaN), so
    # B must sit in [max(S)-80, min_i(rowmax_i)+85]. Row maxes are spread by
    # the per-row c.w1 offset, so estimate them with a q-subsample GEMM and
    # center B in the window.
    cw1 = context @ w1
    q2 = question @ w2
    nq = question.shape[0]
    qsub = question[::16]
    tmax = ((context * w3[None, :]) @ qsub.T + (qsub @ w2)[None, :]).max(axis=1)
    rm = cw1 + tmax
    B = float(0.5 * (rm.max() + rm.min() + 20.0))

    qaugTa = np.empty((102, nq), np.float32)
    qaugTa[0:D] = (question * w3[None, :]).T
    qaugTa[D] = q2
    qaugTa[D + 1] = 1.0
    qaugTa = np.ascontiguousarray(qaugTa)
    import ml_dtypes
    qbf = np.ascontiguousarray(question.astype(ml_dtypes.bfloat16))
    ident = np.eye(P, dtype=np.float32)

    in_maps = []
    for k in range(N_CORES):
        cshard = np.ascontiguousarray(context[k * R:(k + 1) * R])
        ctxTa = np.empty((102, R), np.float32)
        ctxTa[0:D] = cshard.T
        ctxTa[D] = 1.0
        ctxTa[D + 1] = cw1[k * R:(k + 1) * R] - B
        in_maps.append({
            "ctx": cshard,
            "ctxTa": np.ascontiguousarray(ctxTa),
            "qaugTa": qaugTa,
            "qnr": qbf,
            "ident": ident,
        })
    return in_maps


def kernel(**inputs):
    from concourse.bass_utils import run_bass_kernel_spmd

    in_maps = _prep_inputs(inputs)
    res = run_bass_kernel_spmd(_get_nc(), in_maps,
                               core_ids=list(range(N_CORES)))
    return np.concatenate([res.results[k]["g"] for k in range(N_CORES)],
                          axis=0)


def kernel_traced(**inputs):
    """Like kernel() but also returns HW exec time in ns (NTFF profile)."""
    from concourse.bass_utils import run_bass_kernel_spmd

    kernel(**inputs)  # warm compile via cached nc
    in_maps = _prep_inputs(inputs)
    res = run_bass_kernel_spmd(_get_nc(), in_maps,
                               core_ids=list(range(N_CORES)), trace=True)
    out = np.concatenate([res.results[k]["g"] for k in range(N_CORES)],
                         axis=0)
    return out, res
